# revision 4
# baseline (speedup 1.0000x reference)
"""Trainium2 Bass kernel for nn_Attention (Bahdanau-style attention).

Math (per batch row b):
    energy  = tanh(h[b] @ Wh + enc[b] @ We + ba)        # [S, H]
    scores  = energy @ v                                 # [S]
    attn    = softmax(scores)                            # [S]
    context = attn @ enc[b]                              # [2H]

Sharding: data-parallel over batch B=32 across 8 cores (4 rows/core).
Device layout strategy:
  - The big matmul enc @ We contracts over e (=2H): the PE contracts over
    the partition dim, so enc must be laid out [e, s].  We pre-transpose
    enc on the host and upload encT (bf16) per core; a second natural
    copy (bf16) is uploaded for the final context bmm which contracts
    over s.  All accumulation on-device is fp32 (PSUM), so bf16 only
    affects input rounding (~1e-3 rel err).
  - energyT tiles come out [h, s] so the per-row bias (h@Wh + ba) is a
    per-partition scalar -> fused into the tanh activation on ScalarE.
  - scoresT [s_part, 1] columns are produced directly by PE matmuls
    (lhsT = energyT tile, rhs = v chunk), giving softmax the s-on-
    partitions layout that the context bmm needs for its lhsT.
  - softmax skips max-subtraction: |scores| <= ||v||_1 ~ 11.4, exp is
    safe in fp32.
"""

import os

import numpy as np
import ml_dtypes

B, S, H = 32, 2048, 512
E = 2 * H          # 1024
NCORES = 8
BL = B // NCORES   # 4 batch rows per core
P = 128
ST = 512           # s-tile (one PSUM bank of fp32)
NST = S // ST      # 4
EC = E // P        # 8 e-chunks (contraction of the big matmul)
HC = H // P        # 4 h-chunks
SC = S // P        # 16 s-chunks of 128

_BF16 = ml_dtypes.bfloat16

_PROG = None       # cached Bass program
LAST_RESULT = None # BassKernelResults of the last kernel() call (for test.py)


def _emit(ctx, tc, t):
    """Emit the per-core SPMD program. `t` is the dict of dram APs."""
    import concourse.bass as bass  # noqa: F401
    import concourse.mybir as mybir

    nc = tc.nc
    f32 = mybir.dt.float32
    bf16 = mybir.dt.bfloat16
    Tanh = mybir.ActivationFunctionType.Tanh
    Exp = mybir.ActivationFunctionType.Exp
    X = mybir.AxisListType.X

    consts = ctx.enter_context(tc.tile_pool(name="consts", bufs=1))
    io_encn = ctx.enter_context(tc.tile_pool(name="io_encn", bufs=2))
    io_enct = ctx.enter_context(tc.tile_pool(name="io_enct", bufs=3))
    work = ctx.enter_context(tc.tile_pool(name="work", bufs=2))
    ps_energy = ctx.enter_context(tc.tile_pool(name="ps_energy", bufs=2, space="PSUM"))
    ps_sc = ctx.enter_context(tc.tile_pool(name="ps_sc", bufs=2, space="PSUM"))
    ps_misc = ctx.enter_context(tc.tile_pool(name="ps_misc", bufs=2, space="PSUM"))
    ps_ctx = ctx.enter_context(tc.tile_pool(name="ps_ctx", bufs=1, space="PSUM"))

    # ---- constants / weights (scalar-engine DMA queue; enc stream uses sync) ----
    we_sb = consts.tile([P, EC, H], bf16)
    nc.scalar.dma_start(we_sb[:], t["we"].rearrange("(ec p) h -> p ec h", p=P))
    wh_sb = consts.tile([P, HC, H], f32)
    nc.scalar.dma_start(wh_sb[:], t["wh"].rearrange("(ec p) h -> p ec h", p=P))
    ht_sb = consts.tile([P, HC, BL], f32)
    nc.scalar.dma_start(ht_sb[:], t["ht"].rearrange("(ec p) b -> p ec b", p=P))
    ba_sb = consts.tile([P, HC * BL], f32)
    nc.scalar.dma_start(ba_sb[:], t["barep"])
    vt_sb = consts.tile([P, HC, 1], bf16)
    nc.scalar.dma_start(vt_sb[:], t["vt"].rearrange("(hc p) o -> p hc o", p=P))
    ones_row = consts.tile([1, P], f32)
    nc.scalar.dma_start(ones_row[:], t["ones_row"])
    ones_col = consts.tile([P, 1], f32)
    nc.scalar.dma_start(ones_col[:], t["ones_col"])
    id128 = consts.tile([P, P], f32)
    nc.scalar.dma_start(id128[:], t["id128"])

    # ---- h_proj: biasT[h, (hc,b)] = (h @ Wh).T + ba  ------------------------
    misc0 = ps_misc.tile([P, 160], f32, tag="misc")
    for hc in range(HC):
        for ec in range(HC):
            nc.tensor.matmul(
                misc0[:, hc * BL:(hc + 1) * BL],
                wh_sb[:, ec, hc * P:(hc + 1) * P],
                ht_sb[:, ec, :],
                start=(ec == 0),
                stop=(ec == HC - 1),
            )
    bias_sb = consts.tile([P, HC * BL], f32)
    nc.vector.tensor_add(bias_sb[:], misc0[:, 0:HC * BL], ba_sb[:])

    encn_tiles = [None] * BL
    attnT_tiles = [None] * BL
    eng_tiles = [None] * NST
    scT_ps = [None] * BL

    encT_r = t["enct"].rearrange("b (ec p) s -> b p ec s", p=P)
    encN_r = t["encn"].rearrange("b (sc p) e -> b p sc e", p=P)
    attn_r = t["attn_out"].rearrange("b (q p) -> b q p", p=P)

    def emit_scT(b, st):
        # scoresT columns for s-chunks of tile st: [128s, 1] each.
        for j in range(NST):
            sc = st * NST + j
            for hc in range(HC):
                nc.tensor.matmul(
                    scT_ps[b][:, sc:sc + 1],
                    eng_tiles[st][:, hc, j * P:(j + 1) * P],
                    vt_sb[:, hc, :],
                    start=(hc == 0),
                    stop=(hc == HC - 1),
                )

    def emit_softmax(b):
        expT = work.tile([P, SC], f32, tag="expT")
        nc.scalar.activation(expT[:], scT_ps[b][:], Exp)
        misc = ps_misc.tile([P, 160], f32, tag="misc")
        # partition-sum of expT -> [1, 16]
        nc.tensor.matmul(misc[0:1, 0:SC], ones_col[:], expT[:], start=True, stop=True)
        ssum = work.tile([1, 1], f32, tag="ssum")
        nc.vector.reduce_sum(ssum[:], misc[0:1, 0:SC], axis=X)
        sinv = work.tile([1, 1], f32, tag="sinv")
        nc.vector.reciprocal(sinv[:], ssum[:])
        # broadcast 1/sum to all 128 partitions
        nc.tensor.matmul(misc[:, 16:17], ones_row[:], sinv[:], start=True, stop=True)
        inv_col = work.tile([P, 1], f32, tag="inv_col")
        nc.vector.tensor_copy(inv_col[:], misc[:, 16:17])
        attnT_f = work.tile([P, SC], f32, tag="attnT_f")
        nc.vector.tensor_scalar_mul(attnT_f[:], expT[:], inv_col[:])
        attnT_b = work.tile([P, SC], bf16, tag="attnT_b")
        nc.vector.tensor_copy(attnT_b[:], attnT_f[:])
        attnT_tiles[b] = attnT_b
        # attn output row: transpose [128, 16] -> [16, 128]
        nc.tensor.transpose(misc[0:SC, 17:17 + P], attnT_f[:], id128[:])
        att_sb = work.tile([SC, P], f32, tag="att_sb")
        nc.vector.tensor_copy(att_sb[:], misc[0:SC, 17:17 + P])
        nc.sync.dma_start(attn_r[b], att_sb[:])

    def emit_bmm2(b):
        cps = ps_ctx.tile([1, E], f32, tag="cps")
        for sc in range(SC):
            for eh in range(2):
                nc.tensor.matmul(
                    cps[:, eh * ST:(eh + 1) * ST],
                    attnT_tiles[b][:, sc:sc + 1],
                    encn_tiles[b][:, sc, eh * ST:(eh + 1) * ST],
                    start=(sc == 0),
                    stop=(sc == SC - 1),
                )
        csb = work.tile([1, E], f32, tag="csb")
        nc.vector.tensor_copy(csb[:], cps[:])
        nc.sync.dma_start(t["ctx_out"][b:b + 1, :], csb[:])

    for b in range(BL):
        encn_t = io_encn.tile([P, SC, E], bf16, tag="encn")
        nc.scalar.dma_start(encn_t[:], encN_r[b])
        encn_tiles[b] = encn_t
        scT_ps[b] = ps_sc.tile([P, SC], f32, tag="scT", name=f"scT{b}")
        for st in range(NST):
            et = io_enct.tile([P, EC, ST], bf16, tag="et")
            nc.sync.dma_start(et[:], encT_r[b, :, :, st * ST:(st + 1) * ST])
            eng = work.tile([P, HC, ST], bf16, tag="eng")
            for hc in range(HC):
                pe = ps_energy.tile([P, ST], f32, tag="pe")
                for ec in range(EC):
                    nc.tensor.matmul(
                        pe[:],
                        we_sb[:, ec, hc * P:(hc + 1) * P],
                        et[:, ec, :],
                        start=(ec == 0),
                        stop=(ec == EC - 1),
                    )
                nc.scalar.activation(
                    eng[:, hc, :], pe[:], Tanh,
                    bias=bias_sb[:, hc * BL + b:hc * BL + b + 1],
                )
            eng_tiles[st] = eng
            if st > 0:
                emit_scT(b, st - 1)
        # overlap previous row's context bmm with this row's softmax tail
        if b > 0:
            emit_bmm2(b - 1)
        emit_scT(b, NST - 1)
        emit_softmax(b)
    emit_bmm2(BL - 1)


def _build_program():
    from contextlib import ExitStack

    import concourse.mybir as mybir
    import concourse.tile as tile
    from concourse import bacc

    f32 = mybir.dt.float32
    bf16 = mybir.dt.bfloat16

    nc = bacc.Bacc("TRN2", debug=False, num_devices=NCORES)
    t = {
        "enct": nc.dram_tensor("enct", [BL, E, S], bf16, kind="ExternalInput").ap(),
        "encn": nc.dram_tensor("encn", [BL, S, E], bf16, kind="ExternalInput").ap(),
        "we": nc.dram_tensor("we", [E, H], bf16, kind="ExternalInput").ap(),
        "wh": nc.dram_tensor("wh", [H, H], f32, kind="ExternalInput").ap(),
        "ht": nc.dram_tensor("ht", [H, BL], f32, kind="ExternalInput").ap(),
        "barep": nc.dram_tensor("barep", [P, HC * BL], f32, kind="ExternalInput").ap(),
        "vt": nc.dram_tensor("vt", [H, 1], bf16, kind="ExternalInput").ap(),
        "ones_row": nc.dram_tensor("ones_row", [1, P], f32, kind="ExternalInput").ap(),
        "ones_col": nc.dram_tensor("ones_col", [P, 1], f32, kind="ExternalInput").ap(),
        "id128": nc.dram_tensor("id128", [P, P], f32, kind="ExternalInput").ap(),
        "ctx_out": nc.dram_tensor("ctx_out", [BL, E], f32, kind="ExternalOutput").ap(),
        "attn_out": nc.dram_tensor("attn_out", [BL, S], f32, kind="ExternalOutput").ap(),
    }
    with tile.TileContext(nc) as tc, ExitStack() as ctx:
        _emit(ctx, tc, t)
    nc.compile()
    return nc


def get_program():
    global _PROG
    if _PROG is None:
        _PROG = _build_program()
    return _PROG


def make_in_maps(hidden, encoder_outputs, Wa, ba, v):
    hidden = np.asarray(hidden, dtype=np.float32)
    enc = np.asarray(encoder_outputs, dtype=np.float32)
    Wa = np.asarray(Wa, dtype=np.float32)
    ba = np.asarray(ba, dtype=np.float32)
    v = np.asarray(v, dtype=np.float32)

    h = hidden[0]                       # [B, H]
    we = np.ascontiguousarray(Wa[H:]).astype(_BF16)         # [E, H]
    wh = np.ascontiguousarray(Wa[:H])                       # [H, H] f32
    ba_rep = np.ascontiguousarray(
        np.repeat(ba.reshape(HC, P).T, BL, axis=1)          # [128, HC*BL]
    ).astype(np.float32)
    vt = np.ascontiguousarray(v.reshape(H, 1)).astype(_BF16)
    ones_row = np.ones((1, P), np.float32)
    ones_col = np.ones((P, 1), np.float32)
    id128 = np.eye(P, dtype=np.float32)

    in_maps = []
    for c in range(NCORES):
        rows = slice(c * BL, (c + 1) * BL)
        encc = enc[rows]
        in_maps.append({
            "enct": np.ascontiguousarray(encc.transpose(0, 2, 1)).astype(_BF16),
            "encn": encc.astype(_BF16),
            "we": we,
            "wh": wh,
            "ht": np.ascontiguousarray(h[rows].T).astype(np.float32),
            "barep": ba_rep,
            "vt": vt,
            "ones_row": ones_row,
            "ones_col": ones_col,
            "id128": id128,
        })
    return in_maps


def kernel(hidden, encoder_outputs, Wa, ba, v):
    global LAST_RESULT
    from concourse import bass_utils

    nc = get_program()
    in_maps = make_in_maps(hidden, encoder_outputs, Wa, ba, v)
    trace = bool(int(os.environ.get("BASS_TRACE", "0") or "0"))
    res = bass_utils.run_bass_kernel_spmd(
        nc, in_maps, core_ids=list(range(NCORES)), trace=trace
    )
    LAST_RESULT = res
    context = np.concatenate([r["ctx_out"] for r in res.results], axis=0)
    attn = np.concatenate([r["attn_out"] for r in res.results], axis=0)
    return context, attn
